# revision 4
# baseline (speedup 1.0000x reference)
"""CQC contrastive loss kernel for 8 Trainium2 NeuronCores.

Math (B=4096, D=256, TAU=0.5, N=2B=8192):
    x  = concat(Xa, Za)                      [N, D]
    xn = x / ||x||                           (row-normalized)
    S  = xn @ xn.T                           [N, N]
    loss_i = log(sum_{j != i} exp(S_ij/TAU)) - S[i, i+-B]/TAU
    loss   = mean_i loss_i

Sharding: data-parallel over rows. Core c owns rows [1024c, 1024c+1024).
Each core receives X *rotated* by -1024c rows so its rows sit at positions
0..1023 — this makes all SBUF addressing static (one SPMD NEFF for all
cores). The row sum over all columns is permutation-invariant, the diagonal
term is computed analytically from ||xn_i||^2, and the positive pair is a
row-wise dot product against a per-core partner slab input, so nothing else
depends on the rotation.

Per-core pipeline:
    phase 0: load X (rotated), squares+row-reduce via scalar_tensor_tensor
             (fused accum), rsqrt via bit-trick + 3 Newton steps (DVE only,
             keeps ScalarE free for exp), prescale rows, transpose via PE
             into xnT [D, N] (column-normalized), rounded to fp32r.
    main:    for each 128-row block b and 2048-col group g: 8 matmuls
             (fp32r, full PE rate) accumulate S tile in PSUM [128, 2048],
             ScalarE computes exp(2*S) with fused row-sum (accum_out).
    finals:  loss_row = log(rowsum - exp(2*||xn||^2)) - 2*pos, DMA out
             [128, 8] per core; host sums and divides by N.
"""

import numpy as np

import concourse.bacc as bacc
import concourse.tile as tile
from concourse import mybir
from concourse.bass_utils import run_bass_kernel_spmd

F32 = mybir.dt.float32
I32 = mybir.dt.int32
BF16 = mybir.dt.bfloat16
F32R = mybir.dt.float32r
AL = mybir.AluOpType
AF = mybir.ActivationFunctionType

B = 4096
D = 256
N = 2 * B
TAU = 0.5
NCORES = 8
RPC = N // NCORES          # rows per core = 1024
NBLK = RPC // 128          # 128-row blocks per core = 8
NT = N // 128              # x-tiles total = 64
GRP = 4                    # phase-0 groups (16 tiles each)
TPG = NT // GRP            # tiles per group = 16
JG = 4                     # main-loop col groups of 2048
QPG = 4                    # 512-col chunks per col group

# matmul dtype: float32r = fp32 data, PE runs at full (bf16) rate with
# ~tf32 multiply precision (measured 5e-5 rel err on S, ~1e-7 on the loss).
MM_DT = F32R
NAT_DT = F32               # natural-layout prescaled dtype (transpose input)

MAGIC = 0x5F3759DF


def _emit_rsqrt(nc, pool, nsq, rnorm, c0, c1):
    """rnorm[:, c0:c1] = 1/sqrt(nsq[:, c0:c1]) via bit trick + 3 Newton."""
    w = c1 - c0
    x = nsq[:, c0:c1]
    yi = pool.tile([128, w], I32, tag="rs_yi")
    nc.vector.tensor_scalar(out=yi, in0=x.bitcast(I32), scalar1=1,
                            scalar2=None, op0=AL.logical_shift_right)
    nc.vector.tensor_scalar(out=yi, in0=yi, scalar1=MAGIC, scalar2=-1,
                            op0=AL.subtract, op1=AL.mult)
    y = pool.tile([128, w], F32, tag="rs_y")
    nc.vector.tensor_copy(y, yi.bitcast(F32))
    t = pool.tile([128, w], F32, tag="rs_t")
    for it in range(3):
        nc.vector.tensor_mul(t, y, y)
        nc.vector.tensor_mul(t, t, x)
        nc.vector.tensor_scalar(out=t, in0=t, scalar1=-0.5, scalar2=1.5,
                                op0=AL.mult, op1=AL.add)
        dst = rnorm[:, c0:c1] if it == 2 else y
        nc.vector.tensor_mul(dst, y, t)


def build():
    nc = bacc.Bacc("TRN2", target_bir_lowering=False, debug=False,
                   num_devices=NCORES)

    X = nc.dram_tensor("X", [N, D], F32, kind="ExternalInput").ap()
    Xp = nc.dram_tensor("Xp", [RPC, D], F32, kind="ExternalInput").ap()
    ident = nc.dram_tensor("ident", [128, 128], NAT_DT,
                           kind="ExternalInput").ap()
    oLoss = nc.dram_tensor("loss", [128, NBLK], F32,
                           kind="ExternalOutput").ap()

    Xt = X.rearrange("(t p) d -> p t d", p=128)      # [128, 64, 256]
    Xpt = Xp.rearrange("(t p) d -> p t d", p=128)    # [128, 8, 256]

    with tile.TileContext(nc) as tc:
        with (
            tc.tile_pool(name="stream", bufs=2) as st,
            tc.tile_pool(name="persist", bufs=1) as pr,
            tc.tile_pool(name="psum", bufs=2, space="PSUM") as ps,
        ):
            idt = pr.tile([128, 128], NAT_DT, tag="ident")
            nc.sync.dma_start(out=idt, in_=ident)

            nsq = pr.tile([128, NT + NBLK], F32, tag="nsq")
            rnorm = pr.tile([128, NT + NBLK], F32, tag="rnorm")
            rs_parts = pr.tile([128, NBLK * JG], F32, tag="rsp")
            sdiag = pr.tile([128, NBLK], F32, tag="sdiag")
            posd = pr.tile([128, NBLK], F32, tag="posd")

            # xnT[k][g]: [128, 2048] fp32r, d-half k, col group g
            xnT = [[pr.tile([128, TPG * 128], MM_DT, tag=f"xnT{k}_{g}",
                            name=f"xnT{k}_{g}")
                    for g in range(GRP)] for k in range(2)]

            # --- xpart mini phase-0 (no transpose needed) ---
            xp = pr.tile([128, NBLK, D], F32, tag="xp")
            nc.sync.dma_start(out=xp, in_=Xpt)
            for t in range(NBLK):
                scr = st.tile([128, D], F32, tag="sq")
                nc.vector.scalar_tensor_tensor(
                    out=scr, in0=xp[:, t, :], scalar=1.0, in1=xp[:, t, :],
                    op0=AL.mult, op1=AL.mult,
                    accum_out=nsq[:, NT + t:NT + t + 1])
            _emit_rsqrt(nc, st, nsq, rnorm, NT, NT + NBLK)
            xpn = pr.tile([128, NBLK, D], NAT_DT, tag="xpn")
            for t in range(NBLK):
                nc.vector.tensor_scalar_mul(
                    out=xpn[:, t, :], in0=xp[:, t, :],
                    scalar1=rnorm[:, NT + t:NT + t + 1])

            xn_rows = pr.tile([128, TPG, D], NAT_DT, tag="xn_rows")

            def phase0(g):
                xg = st.tile([128, TPG, D], F32, tag="xg")
                nc.sync.dma_start(out=xg, in_=Xt[:, g * TPG:(g + 1) * TPG, :])
                for t in range(TPG):
                    c = g * TPG + t
                    scr = st.tile([128, D], F32, tag="sq")
                    nc.vector.scalar_tensor_tensor(
                        out=scr, in0=xg[:, t, :], scalar=1.0, in1=xg[:, t, :],
                        op0=AL.mult, op1=AL.mult,
                        accum_out=nsq[:, c:c + 1])
                _emit_rsqrt(nc, st, nsq, rnorm, g * TPG, (g + 1) * TPG)
                xn = xn_rows if g == 0 else st.tile([128, TPG, D], NAT_DT,
                                                    tag="xn", name="xn")
                for t in range(TPG):
                    c = g * TPG + t
                    nc.vector.tensor_scalar_mul(
                        out=xn[:, t, :], in0=xg[:, t, :],
                        scalar1=rnorm[:, c:c + 1])
                for k in range(2):
                    pt = ps.tile([128, TPG * 128], NAT_DT, tag="big")
                    for t in range(TPG):
                        nc.tensor.transpose(
                            pt[:, t * 128:(t + 1) * 128],
                            xn[:, t, k * 128:(k + 1) * 128], idt)
                    nc.vector.tensor_copy(xnT[k][g], pt)

            def main_g(g):
                for b in range(NBLK):
                    pm = ps.tile([128, QPG * 512], F32, tag="big")
                    for k in range(2):
                        lhsT = xnT[k][0][:, b * 128:(b + 1) * 128]
                        for q in range(QPG):
                            nc.tensor.matmul(
                                pm[:, q * 512:(q + 1) * 512], lhsT,
                                xnT[k][g][:, q * 512:(q + 1) * 512],
                                start=(k == 0), stop=(k == 1))
                    escr = st.tile([128, QPG * 512], BF16, tag="exps")
                    col = b * JG + g
                    nc.scalar.activation(
                        out=escr, in_=pm, func=AF.Exp, scale=2.0,
                        accum_out=rs_parts[:, col:col + 1])

            phase0(0)
            # sdiag / pos from the normalized natural-layout row blocks
            for t in range(NBLK):
                scr = st.tile([128, D], NAT_DT, tag="sq2")
                nc.vector.scalar_tensor_tensor(
                    out=scr, in0=xn_rows[:, t, :], scalar=1.0,
                    in1=xn_rows[:, t, :], op0=AL.mult, op1=AL.mult,
                    accum_out=sdiag[:, t:t + 1])
                scr2 = st.tile([128, D], NAT_DT, tag="sq2")
                nc.vector.scalar_tensor_tensor(
                    out=scr2, in0=xn_rows[:, t, :], scalar=1.0,
                    in1=xpn[:, t, :], op0=AL.mult, op1=AL.mult,
                    accum_out=posd[:, t:t + 1])
            main_g(0)
            for g in range(1, GRP):
                phase0(g)
                main_g(g)

            # --- finals ---
            rs_tot = pr.tile([128, NBLK], F32, tag="rs_tot")
            nc.vector.tensor_reduce(
                out=rs_tot,
                in_=rs_parts.rearrange("p (b g) -> p b g", g=JG),
                op=AL.add, axis=mybir.AxisListType.X)
            e_diag = pr.tile([128, NBLK], F32, tag="e_diag")
            nc.scalar.activation(out=e_diag, in_=sdiag, func=AF.Exp,
                                 scale=2.0)
            rsm = pr.tile([128, NBLK], F32, tag="rsm")
            nc.vector.tensor_sub(rsm, rs_tot, e_diag)
            lg = pr.tile([128, NBLK], F32, tag="lg")
            nc.scalar.activation(out=lg, in_=rsm, func=AF.Ln)
            lt = pr.tile([128, NBLK], F32, tag="lt")
            nc.vector.scalar_tensor_tensor(
                out=lt, in0=posd, scalar=-2.0, in1=lg,
                op0=AL.mult, op1=AL.add)
            nc.sync.dma_start(out=oLoss, in_=lt)

    nc.finalize()
    return nc


_NC_CACHE = {}
last_results = None


def kernel(Xa: np.ndarray, Za: np.ndarray) -> np.ndarray:
    global last_results
    if "nc" not in _NC_CACHE:
        _NC_CACHE["nc"] = build()
    nc = _NC_CACHE["nc"]

    X = np.ascontiguousarray(np.concatenate([Xa, Za], axis=0),
                             dtype=np.float32)
    ident = np.eye(128, dtype=mybir.dt.np(NAT_DT))
    in_maps = []
    for c in range(NCORES):
        r = RPC * c
        Xrot = np.ascontiguousarray(np.concatenate([X[r:], X[:r]], axis=0))
        p = (r + B) % N
        Xpart = np.ascontiguousarray(X[p:p + RPC])
        in_maps.append({"X": Xrot, "Xp": Xpart, "ident": ident})

    last_results = run_bass_kernel_spmd(nc, in_maps,
                                        core_ids=list(range(NCORES)))
    total = 0.0
    for r in last_results.results:
        total += r["loss"].astype(np.float64).sum()
    return np.float32(total / N)


# revision 8
# speedup vs baseline: 1.4396x; 1.4396x over previous
"""CQC contrastive loss kernel for 8 Trainium2 NeuronCores.

Math (B=4096, D=256, TAU=0.5, N=2B=8192):
    x  = concat(Xa, Za)                      [N, D]
    xn = x / ||x||                           (row-normalized)
    S  = xn @ xn.T                           [N, N]
    loss_i = log(sum_{j != i} exp(S_ij/TAU)) - S[i, i+-B]/TAU
    loss   = mean_i loss_i

Sharding: data-parallel over rows. Core c owns rows [1024c, 1024c+1024).
Each core receives X *rotated* by -1024c rows so its rows sit at positions
0..1023 — all SBUF addressing is static (one SPMD NEFF for all cores). The
row sum over all columns is permutation-invariant, the diagonal term is
computed from ||xn_i||^2 of the same on-chip data, and the positive pair is
a row-wise dot against a per-core partner-slab input, so nothing else
depends on the rotation. Inputs are pre-cast to bf16 on the host (the
matmul runs in bf16 anyway; norms/statistics accumulate in fp32 on-chip).

Per-core pipeline:
    phase 0 (per 8-tile group): DMA load, squares+row-sum via
        scalar_tensor_tensor (fused fp32 accum), rsqrt via bit-trick +
        3 Newton steps (DVE-only, keeps ScalarE free for exp), per-row
        prescale, PE transpose (bf16, 1 cyc/row) into a dedicated 1-bank
        PSUM tile, DVE copy into xnT [D, N] (column-normalized bf16).
    main (per 128-row block b, chunk group of <=3 512-col chunks): bf16
        matmuls accumulate S in a 3-bank PSUM tile (full PE rate), ScalarE
        computes exp(2*S) with fused row-sum (accum_out) — nothing else
        reads S. Chunk groups are aligned so each one only depends on
        phase-0 groups that are already flowing.
    finals: loss_row = log(rowsum - exp(2*||xn||^2)) - 2*pos, DMA out
        [128, 8] per core; host sums in float64 and divides by N.
"""

import numpy as np
import ml_dtypes

import concourse.bacc as bacc
import concourse.tile as tile
from concourse import mybir
from concourse.bass_utils import run_bass_kernel_spmd

F32 = mybir.dt.float32
I32 = mybir.dt.int32
BF16 = mybir.dt.bfloat16
AL = mybir.AluOpType
AF = mybir.ActivationFunctionType

B = 4096
D = 256
N = 2 * B
TAU = 0.5
NCORES = 8
RPC = N // NCORES          # rows per core = 1024
NBLK = RPC // 128          # 128-row blocks per core = 8
NT = N // 128              # x-tiles total = 64
GRP = 8                    # phase-0 groups (8 tiles each)
TPG = NT // GRP            # tiles per group = 8
# main-loop chunk groups (in 512-col units), sized to fit a 3-bank PSUM
# tile and aligned so each group only needs phase-0 groups already emitted
CGS = [(0, 1, 2), (3, 4, 5), (6, 7, 8), (9, 10, 11), (12, 13), (14, 15)]
NCG = len(CGS)

MAGIC = 0x5F3759DF


def _emit_rsqrt(nc, pool, nsq, rnorm, c0, c1):
    """rnorm[:, c0:c1] = 1/sqrt(nsq[:, c0:c1]) via bit trick + 3 Newton."""
    w = c1 - c0
    x = nsq[:, c0:c1]
    yi = pool.tile([128, w], I32, tag="rs_yi", name="rs_yi")
    nc.vector.tensor_scalar(out=yi, in0=x.bitcast(I32), scalar1=1,
                            scalar2=None, op0=AL.logical_shift_right)
    nc.vector.tensor_scalar(out=yi, in0=yi, scalar1=MAGIC, scalar2=-1,
                            op0=AL.subtract, op1=AL.mult)
    y = pool.tile([128, w], F32, tag="rs_y", name="rs_y")
    nc.vector.tensor_copy(y, yi.bitcast(F32))
    t = pool.tile([128, w], F32, tag="rs_t", name="rs_t")
    for it in range(3):
        nc.vector.tensor_mul(t, y, y)
        nc.vector.tensor_mul(t, t, x)
        nc.vector.tensor_scalar(out=t, in0=t, scalar1=-0.5, scalar2=1.5,
                                op0=AL.mult, op1=AL.add)
        dst = rnorm[:, c0:c1] if it == 2 else y
        nc.vector.tensor_mul(dst, y, t)


def _patch_act_tables():
    """Force every activation onto the one table set that covers both exp
    and ln (plus copy/square/identity fillers), so the kernel pays a single
    ACT table load instead of three. Indices of the other sets are kept
    (emptied, not removed) because act_func_set_id is a positional index
    into act_info.json."""
    if getattr(bacc, "_cqc_act_patch", False):
        return
    orig = bacc.get_activation_tables

    def patched(module_arch):
        tabs = orig(module_arch)
        keep = "natural_log_exp_and_others"
        if keep in tabs:
            tabs = {name: (fns if name == keep else set())
                    for name, fns in tabs.items()}
        return tabs

    bacc.get_activation_tables = patched
    bacc._cqc_act_patch = True


def build():
    _patch_act_tables()
    nc = bacc.Bacc("TRN2", target_bir_lowering=False, debug=False,
                   num_devices=NCORES)

    X = nc.dram_tensor("X", [N, D], BF16, kind="ExternalInput").ap()
    Xp = nc.dram_tensor("Xp", [RPC, D], BF16, kind="ExternalInput").ap()
    ident = nc.dram_tensor("ident", [128, 128], BF16,
                           kind="ExternalInput").ap()
    oLoss = nc.dram_tensor("loss", [128, NBLK], F32,
                           kind="ExternalOutput").ap()

    Xt = X.rearrange("(t p) d -> p t d", p=128)      # [128, 64, 256]
    Xpt = Xp.rearrange("(t p) d -> p t d", p=128)    # [128, 8, 256]

    with tile.TileContext(nc) as tc:
        with (
            tc.tile_pool(name="stream", bufs=3) as st,
            tc.tile_pool(name="persist", bufs=1) as pr,
            tc.tile_pool(name="psum", bufs=2, space="PSUM") as ps,
        ):
            idt = pr.tile([128, 128], BF16, tag="ident")
            nc.sync.dma_start(out=idt, in_=ident)

            # Preload the ln table set while everything waits on DMA.
            one = pr.tile([128, 1], F32, tag="one")
            nc.gpsimd.memset(one, 1.0)
            lnscr = pr.tile([128, 1], F32, tag="lnscr")
            nc.scalar.activation(out=lnscr, in_=one, func=AF.Ln)

            nsq = pr.tile([128, NT + NBLK], F32, tag="nsq")
            rnorm = pr.tile([128, NT + NBLK], F32, tag="rnorm")
            rs_parts = pr.tile([128, NBLK * NCG], F32, tag="rsp")
            sdiag = pr.tile([128, NBLK], F32, tag="sdiag")
            posd = pr.tile([128, NBLK], F32, tag="posd")

            # xnT[k][g]: [128, 1024] bf16 — d-half k, 1024-col group g
            xnT = [[pr.tile([128, TPG * 128], BF16, tag=f"xnT{k}_{g}",
                            name=f"xnT{k}_{g}")
                    for g in range(GRP)] for k in range(2)]

            xn_rows = pr.tile([128, TPG, D], BF16, tag="xn_rows")

            def phase0(g):
                xg = st.tile([128, TPG, D], BF16, tag="xg", name="xg")
                nc.sync.dma_start(out=xg, in_=Xt[:, g * TPG:(g + 1) * TPG, :])
                for t in range(TPG):
                    c = g * TPG + t
                    scr = st.tile([128, D], BF16, tag="sq", name="sq")
                    nc.vector.scalar_tensor_tensor(
                        out=scr, in0=xg[:, t, :], scalar=1.0, in1=xg[:, t, :],
                        op0=AL.mult, op1=AL.mult,
                        accum_out=nsq[:, c:c + 1])
                _emit_rsqrt(nc, st, nsq, rnorm, g * TPG, (g + 1) * TPG)
                xn = xn_rows if g == 0 else st.tile([128, TPG, D], BF16,
                                                    tag="xn", name="xn")
                for t in range(TPG):
                    c = g * TPG + t
                    nc.vector.tensor_scalar_mul(
                        out=xn[:, t, :], in0=xg[:, t, :],
                        scalar1=rnorm[:, c:c + 1])
                for k in range(2):
                    pt = ps.tile([128, TPG * 128], BF16, tag="tp",
                                 name="pt")
                    for t in range(TPG):
                        nc.tensor.transpose(
                            pt[:, t * 128:(t + 1) * 128],
                            xn[:, t, k * 128:(k + 1) * 128], idt)
                    nc.vector.tensor_copy(xnT[k][g], pt)

            def main_cg(cgi):
                cg = CGS[cgi]
                w = len(cg) * 512
                for b in range(NBLK):
                    pm = ps.tile([128, w], F32, tag="big", name="pm",
                                 padded_shape=[128, 3 * 512])
                    for k in range(2):
                        lhsT = xnT[k][0][:, b * 128:(b + 1) * 128]
                        for i, c in enumerate(cg):
                            nc.tensor.matmul(
                                pm[:, i * 512:(i + 1) * 512], lhsT,
                                xnT[k][c // 2]
                                   [:, (c % 2) * 512:(c % 2 + 1) * 512],
                                start=(k == 0), stop=(k == 1))
                    escr = st.tile([128, w], BF16, tag="exps", name="exps",
                                   padded_shape=[128, 3 * 512])
                    col = b * NCG + cgi
                    nc.scalar.activation(
                        out=escr, in_=pm, func=AF.Exp, scale=2.0,
                        accum_out=rs_parts[:, col:col + 1])

            def xpart_chain():
                xp = pr.tile([128, NBLK, D], BF16, tag="xp")
                nc.sync.dma_start(out=xp, in_=Xpt)
                for t in range(NBLK):
                    scr = st.tile([128, D], BF16, tag="sq", name="sq")
                    nc.vector.scalar_tensor_tensor(
                        out=scr, in0=xp[:, t, :], scalar=1.0,
                        in1=xp[:, t, :], op0=AL.mult, op1=AL.mult,
                        accum_out=nsq[:, NT + t:NT + t + 1])
                _emit_rsqrt(nc, st, nsq, rnorm, NT, NT + NBLK)
                xpn = pr.tile([128, NBLK, D], BF16, tag="xpn")
                for t in range(NBLK):
                    nc.vector.tensor_scalar_mul(
                        out=xpn[:, t, :], in0=xp[:, t, :],
                        scalar1=rnorm[:, NT + t:NT + t + 1])
                # sdiag / pos from normalized bf16 tiles (matches matmul data)
                for t in range(NBLK):
                    scr = st.tile([128, D], BF16, tag="sq", name="sq")
                    nc.vector.scalar_tensor_tensor(
                        out=scr, in0=xn_rows[:, t, :], scalar=1.0,
                        in1=xn_rows[:, t, :], op0=AL.mult, op1=AL.mult,
                        accum_out=sdiag[:, t:t + 1])
                    scr2 = st.tile([128, D], BF16, tag="sq", name="sq")
                    nc.vector.scalar_tensor_tensor(
                        out=scr2, in0=xn_rows[:, t, :], scalar=1.0,
                        in1=xpn[:, t, :], op0=AL.mult, op1=AL.mult,
                        accum_out=posd[:, t:t + 1])

            phase0(0)
            phase0(1)
            main_cg(0)            # chunks 0-2   (needs g0, g1)
            phase0(2)
            main_cg(1)            # chunks 3-5   (needs g2)
            phase0(3)
            phase0(4)
            main_cg(2)            # chunks 6-8   (needs g3, g4)
            phase0(5)
            main_cg(3)            # chunks 9-11  (needs g5)
            phase0(6)
            main_cg(4)            # chunks 12-13 (needs g6)
            phase0(7)
            main_cg(5)            # chunks 14-15 (needs g7)
            xpart_chain()

            # --- finals ---
            rs_tot = pr.tile([128, NBLK], F32, tag="rs_tot")
            nc.vector.tensor_reduce(
                out=rs_tot,
                in_=rs_parts.rearrange("p (b g) -> p b g", g=NCG),
                op=AL.add, axis=mybir.AxisListType.X)
            e_diag = pr.tile([128, NBLK], F32, tag="e_diag")
            nc.scalar.activation(out=e_diag, in_=sdiag, func=AF.Exp,
                                 scale=2.0)
            rsm = pr.tile([128, NBLK], F32, tag="rsm")
            nc.vector.tensor_sub(rsm, rs_tot, e_diag)
            lg = pr.tile([128, NBLK], F32, tag="lg")
            nc.scalar.activation(out=lg, in_=rsm, func=AF.Ln)
            lt = pr.tile([128, NBLK], F32, tag="lt")
            nc.vector.scalar_tensor_tensor(
                out=lt, in0=posd, scalar=-2.0, in1=lg,
                op0=AL.mult, op1=AL.add)
            nc.sync.dma_start(out=oLoss, in_=lt)

    nc.finalize()
    return nc


_NC_CACHE = {}
last_results = None


def kernel(Xa: np.ndarray, Za: np.ndarray) -> np.ndarray:
    global last_results
    if "nc" not in _NC_CACHE:
        _NC_CACHE["nc"] = build()
    nc = _NC_CACHE["nc"]

    X = np.concatenate([np.asarray(Xa), np.asarray(Za)], axis=0)
    Xb = X.astype(ml_dtypes.bfloat16)
    ident = np.eye(128, dtype=ml_dtypes.bfloat16)
    in_maps = []
    for c in range(NCORES):
        r = RPC * c
        Xrot = np.ascontiguousarray(np.concatenate([Xb[r:], Xb[:r]], axis=0))
        p = (r + B) % N
        Xpart = np.ascontiguousarray(Xb[p:p + RPC])
        in_maps.append({"X": Xrot, "Xp": Xpart, "ident": ident})

    last_results = run_bass_kernel_spmd(nc, in_maps,
                                        core_ids=list(range(NCORES)))
    total = 0.0
    for r in last_results.results:
        total += r["loss"].astype(np.float64).sum()
    return np.float32(total / N)
